# revision 21
# baseline (speedup 1.0000x reference)
"""Multi-head attention (B=2, L=2048, D=1024, H=16) on 8 Trainium2 NeuronCores.

Sharding: core c -> batch b = c//4, head-group g = c%4 (heads 4g..4g+3).
Each core computes, for its batch and 4 heads:
  - partial output slice:  att_norm(heads) @ Wo[:, 256g:256g+256].T   [2048, 1024]
  - sum over its 4 heads of softmax weights                            [2048, 2048]
Host sums partials over the 4 cores of each batch (+bo) and divides the
weight sums by H=16.

Device pipeline per core (all fp32):
  1. PE-transpose Wq/Wk/Wv slices -> WqT/WkT/WvT [1024(8x128), 256];
     Wo slice -> WoT [256(2x128), 1024].
  2. Per 512-row chunk of x: PE-transpose x -> xT [1024(8x128), 512];
     project: QT/KT [256, 2048] (Q scaled by 1/8), V [2048, 4, 64] (+ones col).
  3. Per (head h, q-chunk qc of 1024):
     scores^T tiles [128kpos, 1024q] -> exp (ACT) -> attended accumulation
     with a ones-column in V giving the softmax denominator for free:
       attT_un [64, 1024], denom [1, 1024].
     recip = 1/denom; bounce recip through DRAM to get
       rb [64, 1024] (replicated) and recip_nat [128, 8] -> log -> -lse.
     attT = attT_un * rb  (normalized, stored [256, 2048] across heads).
  4. Natural-layout pass: scores [128q, 2048kpos] -> exp(S - lse) via ACT
     per-partition bias = normalized softmax weights; accumulate 4 heads
     (DVE adds) -> wsum DRAM.
  5. Output projection: out[q, :] = sum_a attT[a, q] * WoT[a, :].
"""

import numpy as np

import concourse.bass as bass
import concourse.mybir as mybir
from concourse import bacc
from concourse.tile import TileContext
from concourse.masks import make_identity

F32 = mybir.dt.float32
N_CORES = 8
B, L, D, H, HD = 2, 2048, 1024, 16, 64
HPC = 4          # heads per core
A = HPC * HD     # 256 attended dims per core
SCALE = HD ** -0.5

AF = mybir.ActivationFunctionType
ALU = mybir.AluOpType

# Matmul input dtype: float32r reads the same fp32 bytes but runs the PE at
# full rate (1 cycle/row vs 4 for true fp32, which is emitted as two
# half-speed hi/lo passes). Set to mybir.dt.float32 for exact fp32.
MM_DT = mybir.dt.float32r


def _mm(nc, out, lhsT, rhs, **kw):
    nc.tensor.matmul(out, lhsT=lhsT, rhs=rhs, **kw)


def build_nc():
    nc = bacc.Bacc(trn_type="TRN2", num_devices=N_CORES, debug=False)

    xq = nc.dram_tensor("xq", (L, D), F32, kind="ExternalInput")
    xk = nc.dram_tensor("xk", (L, D), F32, kind="ExternalInput")
    xv = nc.dram_tensor("xv", (L, D), F32, kind="ExternalInput")
    wq = nc.dram_tensor("wq", (A, D), F32, kind="ExternalInput")
    wk = nc.dram_tensor("wk", (A, D), F32, kind="ExternalInput")
    wv = nc.dram_tensor("wv", (A, D), F32, kind="ExternalInput")
    wo = nc.dram_tensor("wo", (D, A), F32, kind="ExternalInput")

    out_p = nc.dram_tensor("out_p", (L, D), F32, kind="ExternalOutput")
    wsum = nc.dram_tensor("wsum", (L, L), F32, kind="ExternalOutput")

    with TileContext(nc) as tc:
        _build_tile(nc, tc, xq, xk, xv, wq, wk, wv, wo, out_p, wsum)
    nc.compile()
    return nc


def _build_tile(nc, tc, xq, xk, xv, wq, wk, wv, wo, out_p, wsum):
    DK = D // 128          # 8 d-chunks
    LC = L // 512          # 4 l-chunks
    KC = L // 128          # 16 kpos chunks
    QC = L // 1024         # 2 q super-chunks
    QS = 1024 // 128       # 8 q sub chunks per super-chunk

    with tc.tile_pool(name="const", bufs=1) as const_pool:
        ident = const_pool.tile([128, 128], F32)
        make_identity(nc, ident)

        # persistent activations
        qT = const_pool.tile([128, 2, L], MM_DT, tag="qT")      # [a%128, a//128, l]
        kT = const_pool.tile([128, 2, L], MM_DT, tag="kT")
        vv = const_pool.tile([128, KC, HPC, HD + 1], MM_DT, tag="vv")
        attT = const_pool.tile([128, 2, L], MM_DT, tag="attT")
        woT = const_pool.tile([128, 2, D], MM_DT, tag="woT")    # [a%128, a//128, o]

        # ones column for denominators (memset doesn't accept fp32r: write
        # the fp32 bit pattern through a bitcast view; 1.0 is identical)
        nc.any.memset(vv[:, :, :, HD:HD + 1].bitcast(F32), 1.0)

        # ---- Phases W+X: weight/activation transposes + projections ----
        wT_pool = tc.alloc_tile_pool(name="wT", bufs=1)
        wqT = wT_pool.tile([128, DK, A], MM_DT, tag="wqT")      # [d%128, d//128, a]
        wkT = wT_pool.tile([128, DK, A], MM_DT, tag="wkT")
        wvT = wT_pool.tile([128, DK, A], MM_DT, tag="wvT")

        with (
            tc.tile_pool(name="wload", bufs=2) as wload,
            tc.tile_pool(name="wpsum", bufs=2, space="PSUM") as wpsum,
        ):
            for w_dram, wT in ((wq, wqT), (wk, wkT), (wv, wvT)):
                w_sb = wload.tile([128, 2, D], F32, tag="w_sb")
                nc.sync.dma_start(w_sb[:], w_dram.ap().rearrange("(o p) d -> p o d", p=128))
                for dk in range(DK):
                    ps = wpsum.tile([128, 256], F32, tag="wps")
                    for o in range(2):
                        nc.tensor.transpose(
                            ps[:, o * 128:(o + 1) * 128],
                            w_sb[:, o, dk * 128:(dk + 1) * 128],
                            ident,
                        )
                    nc.vector.tensor_copy(wT[:, dk, :], ps[:])
            wo_sb = wload.tile([128, DK, A], F32, tag="wo_sb")
            nc.sync.dma_start(wo_sb[:], wo.ap().rearrange("(o p) a -> p o a", p=128))
            for a in range(2):
                for o4 in range(2):
                    ps = wpsum.tile([128, 4, 128], F32, tag="wops")
                    for oi in range(4):
                        o = o4 * 4 + oi
                        nc.tensor.transpose(
                            ps[:, oi, :],
                            wo_sb[:, o, a * 128:(a + 1) * 128],
                            ident,
                        )
                    nc.vector.tensor_copy(
                        woT[:, a, o4 * 512:(o4 + 1) * 512],
                        ps[:].rearrange("p o i -> p (o i)"),
                    )

        # ---- Phase X: x transposes + projections ----
        with (
            tc.tile_pool(name="xload", bufs=2) as xload,
            tc.tile_pool(name="xtp", bufs=2) as xtp,
            tc.tile_pool(name="xpsum", bufs=3, space="PSUM") as xpsum,
            tc.tile_pool(name="ppsum", bufs=2, space="PSUM") as ppsum,
        ):
            for lc in range(LC):
                lsl = slice(lc * 512, (lc + 1) * 512)
                for ti, (x_dram, kind) in enumerate(((xq, "q"), (xk, "k"), (xv, "v"))):
                    x_sb = xload.tile([128, 4, D], F32, tag="x_sb")
                    nc.sync.dma_start(
                        x_sb[:], x_dram.ap()[lsl, :].rearrange("(ls p) d -> p ls d", p=128)
                    )
                    xT = xtp.tile([128, DK, 512], MM_DT, tag="xT")
                    for dk in range(DK):
                        ps = xpsum.tile([128, 512], F32, tag="xps")
                        for ls in range(4):
                            nc.tensor.transpose(
                                ps[:, ls * 128:(ls + 1) * 128],
                                x_sb[:, ls, dk * 128:(dk + 1) * 128],
                                ident,
                            )
                        nc.vector.tensor_copy(xT[:, dk, :], ps[:])
                    if kind in ("q", "k"):
                        wT = wqT if kind == "q" else wkT
                        dstT = qT if kind == "q" else kT
                        for m in range(2):
                            ps = ppsum.tile([128, 512], F32, tag="pps")
                            for dk in range(DK):
                                _mm(
                                    nc,
                                    ps[:],
                                    wT[:, dk, m * 128:(m + 1) * 128],
                                    xT[:, dk, :],
                                    start=(dk == 0),
                                    stop=(dk == DK - 1),
                                )
                            if kind == "q":
                                nc.vector.tensor_scalar_mul(dstT[:, m, lsl], ps[:], SCALE)
                            else:
                                nc.vector.tensor_copy(dstT[:, m, lsl], ps[:])
                    else:
                        for kp in range(4):
                            kc = lc * 4 + kp
                            ps = ppsum.tile([128, 256], F32, tag="vps")
                            for dk in range(DK):
                                _mm(
                                    nc,
                                    ps[:],
                                    xT[:, dk, kp * 128:(kp + 1) * 128],
                                    wvT[:, dk, :],
                                    start=(dk == 0),
                                    stop=(dk == DK - 1),
                                )
                            nc.vector.tensor_copy(
                                vv[:, kc, :, 0:HD],
                                ps[:].rearrange("p (h d) -> p h d", h=HPC),
                            )
        wT_pool.release()

        # ---- Phase A: attention ----
        with (
            tc.tile_pool(name="spsum", bufs=2, space="PSUM") as spsum,
            tc.tile_pool(name="apsum", bufs=1, space="PSUM") as apsum,
            tc.tile_pool(name="npsum", bufs=1, space="PSUM") as npsum,
            tc.tile_pool(name="epool", bufs=4) as epool,
            tc.tile_pool(name="aupool", bufs=5) as aupool,
            tc.tile_pool(name="small", bufs=2) as small,
            tc.tile_pool(name="dram", bufs=8, space="DRAM") as dram_pool,
            tc.tile_pool(name="wacc", bufs=2) as wacc_pool,
            tc.tile_pool(name="opool", bufs=2) as opool,
        ):
            lse_nat = const_pool.tile([128, HPC, QC, QS], F32, tag="lse_nat")
            for qc in range(QC):
                qsl = slice(qc * 1024, (qc + 1) * 1024)
                att_uns = []
                for h in range(HPC):
                    hr = slice(64 * (h % 2), 64 * (h % 2) + 64)
                    ho = h // 2
                    att_ps = apsum.tile([65, 1024], F32, tag="att")
                    for kc in range(KC):
                        s_ps = spsum.tile([128, 1024], F32, tag="sT")
                        for half in range(2):
                            _mm(
                                nc,
                                s_ps[:, half * 512:(half + 1) * 512],
                                kT[hr, ho, kc * 128:(kc + 1) * 128],
                                qT[hr, ho, qc * 1024 + half * 512: qc * 1024 + (half + 1) * 512],
                                start=True,
                                stop=True,
                            )
                        eT = epool.tile([128, 1024], MM_DT, tag="eT")
                        nc.scalar.activation(eT[:], s_ps[:], AF.Exp)
                        for half in range(2):
                            _mm(
                                nc,
                                att_ps[:, half * 512:(half + 1) * 512],
                                vv[:, kc, h, :],
                                eT[:, half * 512:(half + 1) * 512],
                                start=(kc == 0),
                                stop=(kc == KC - 1),
                            )
                    # evict immediately with a plain copy so att_ps frees for
                    # the next head; the recip/normalize chain runs off the
                    # critical path from SBUF
                    att_un = aupool.tile([65, 1024], F32, tag="att_un")
                    nc.vector.tensor_copy(att_un[:], att_ps[:])
                    att_uns.append(att_un)
                for h in range(HPC):
                    hr = slice(64 * (h % 2), 64 * (h % 2) + 64)
                    ho = h // 2
                    att_un = att_uns[h]
                    # reciprocal must stay on partition 64 (lanes are wired
                    # to partitions; no cross-partition moves on DVE/ACT)
                    recip_f = small.tile([65, 1024], F32, tag="recip_f")
                    nc.vector.reciprocal(recip_f[64:65, :], att_un[64:65, :])
                    scratch = dram_pool.tile([1024], F32, tag="scratch")
                    nc.sync.dma_start(scratch[:][None, :], recip_f[64:65, :])
                    # replicate across 64 partitions
                    rb = small.tile([64, 1024], F32, tag="rb")
                    nc.sync.dma_start(
                        rb[:],
                        scratch[:][None, :].to_broadcast((64, 1024)),
                    )
                    recip_nat = small.tile([128, QS], F32, tag="recip_nat")
                    nc.sync.dma_start(
                        recip_nat[:], scratch[:].rearrange("(s p) -> p s", p=128)
                    )
                    nc.scalar.activation(lse_nat[:, h, qc, :], recip_nat[:], AF.Ln)
                    # normalize attended rows and store to attT; odd heads
                    # land at partitions 64-127, which DVE cannot write from
                    # a base-0 source, so bounce through SBUF via DMA
                    if h % 2 == 0:
                        nc.vector.tensor_tensor(
                            attT[0:64, ho, qsl],
                            att_un[0:64, :],
                            rb[:],
                            ALU.mult,
                        )
                    else:
                        att_tmp = small.tile([64, 1024], MM_DT, tag="att_tmp")
                        nc.vector.tensor_tensor(
                            att_tmp[:],
                            att_un[0:64, :],
                            rb[:],
                            ALU.mult,
                        )
                        nc.sync.dma_start(attT[64:128, ho, qsl], att_tmp[:])

                # natural pass: normalized softmax weights summed over heads
                for qs in range(QS):
                    q0 = qc * 1024 + qs * 128
                    wacc = wacc_pool.tile([128, L], F32, tag="wacc")
                    wtmp = wacc_pool.tile([128, L], F32, tag="wtmp")
                    for h in range(HPC):
                        hr = slice(64 * (h % 2), 64 * (h % 2) + 64)
                        ho = h // 2
                        dst = wacc if h == 0 else wtmp
                        for kh in range(2):
                            n_ps = npsum.tile([128, 1024], F32, tag="nat")
                            for half in range(2):
                                k0 = kh * 1024 + half * 512
                                _mm(
                                    nc,
                                    n_ps[:, half * 512:(half + 1) * 512],
                                    qT[hr, ho, q0:q0 + 128],
                                    kT[hr, ho, k0:k0 + 512],
                                    start=True,
                                    stop=True,
                                )
                            nc.scalar.activation(
                                dst[:, kh * 1024:(kh + 1) * 1024],
                                n_ps[:],
                                AF.Exp,
                                bias=lse_nat[:, h, qc, qs:qs + 1],
                            )
                        if h > 0:
                            nc.vector.tensor_tensor(wacc[:], wacc[:], wtmp[:], ALU.add)
                    nc.sync.dma_start(wsum.ap()[q0:q0 + 128, :], wacc[:])

                # output projection for this q super-chunk
                for qs in range(QS):
                    q0 = qc * 1024 + qs * 128
                    o_ps = npsum.tile([128, 1024], F32, tag="nat")
                    for oh in range(2):
                        for ac in range(2):
                            _mm(
                                nc,
                                o_ps[:, oh * 512:(oh + 1) * 512],
                                attT[:, ac, q0:q0 + 128],
                                woT[:, ac, oh * 512:(oh + 1) * 512],
                                start=(ac == 0),
                                stop=(ac == 1),
                            )
                    o_sb = opool.tile([128, 1024], F32, tag="o_sb")
                    nc.vector.tensor_copy(o_sb[:], o_ps[:])
                    nc.sync.dma_start(out_p.ap()[q0:q0 + 128, :], o_sb[:])


_NC_CACHE = None


def _get_nc():
    global _NC_CACHE
    if _NC_CACHE is None:
        _NC_CACHE = build_nc()
    return _NC_CACHE


def make_in_maps(inputs):
    query, key, value = inputs["query"], inputs["key"], inputs["value"]
    Wq, Wk, Wv, Wo = inputs["Wq"], inputs["Wk"], inputs["Wv"], inputs["Wo"]
    in_maps = []
    for c in range(N_CORES):
        b, g = c // 4, c % 4
        asl = slice(g * A, (g + 1) * A)
        in_maps.append({
            "xq": np.ascontiguousarray(query[b], dtype=np.float32),
            "xk": np.ascontiguousarray(key[b], dtype=np.float32),
            "xv": np.ascontiguousarray(value[b], dtype=np.float32),
            "wq": np.ascontiguousarray(Wq[asl], dtype=np.float32),
            "wk": np.ascontiguousarray(Wk[asl], dtype=np.float32),
            "wv": np.ascontiguousarray(Wv[asl], dtype=np.float32),
            "wo": np.ascontiguousarray(Wo[:, asl], dtype=np.float32),
        })
    return in_maps


def kernel(query, key, value, Wq, bq, Wk, bk, Wv, bv, Wo, bo):
    from concourse.bass_utils import run_bass_kernel_spmd

    nc = _get_nc()
    in_maps = make_in_maps(dict(
        query=np.asarray(query), key=np.asarray(key), value=np.asarray(value),
        Wq=np.asarray(Wq), Wk=np.asarray(Wk), Wv=np.asarray(Wv), Wo=np.asarray(Wo),
    ))
    res = run_bass_kernel_spmd(nc, in_maps, core_ids=list(range(N_CORES)))
    out = np.zeros((B, L, D), np.float32)
    wmean = np.zeros((B, L, L), np.float32)
    for c in range(N_CORES):
        b = c // 4
        out[b] += res.results[c]["out_p"]
        wmean[b] += res.results[c]["wsum"]
    out += np.asarray(bo, np.float32)
    wmean /= H
    return out, wmean


# revision 22
# speedup vs baseline: 1.0494x; 1.0494x over previous
"""Multi-head attention (B=2, L=2048, D=1024, H=16) on 8 Trainium2 NeuronCores.

Sharding: core c -> batch b = c//4, head-group g = c%4 (heads 4g..4g+3).
Each core computes, for its batch and 4 heads:
  - partial output slice:  att_norm(heads) @ Wo[:, 256g:256g+256].T   [2048, 1024]
  - sum over its 4 heads of softmax weights                            [2048, 2048]
Host sums partials over the 4 cores of each batch (+bo) and divides the
weight sums by H=16.

Device pipeline per core (all fp32):
  1. PE-transpose Wq/Wk/Wv slices -> WqT/WkT/WvT [1024(8x128), 256];
     Wo slice -> WoT [256(2x128), 1024].
  2. Per 512-row chunk of x: PE-transpose x -> xT [1024(8x128), 512];
     project: QT/KT [256, 2048] (Q scaled by 1/8), V [2048, 4, 64] (+ones col).
  3. Per (head h, q-chunk qc of 1024):
     scores^T tiles [128kpos, 1024q] -> exp (ACT) -> attended accumulation
     with a ones-column in V giving the softmax denominator for free:
       attT_un [64, 1024], denom [1, 1024].
     recip = 1/denom; bounce recip through DRAM to get
       rb [64, 1024] (replicated) and recip_nat [128, 8] -> log -> -lse.
     attT = attT_un * rb  (normalized, stored [256, 2048] across heads).
  4. Natural-layout pass: scores [128q, 2048kpos] -> exp(S - lse) via ACT
     per-partition bias = normalized softmax weights; accumulate 4 heads
     (DVE adds) -> wsum DRAM.
  5. Output projection: out[q, :] = sum_a attT[a, q] * WoT[a, :].
"""

import numpy as np

import concourse.bass as bass
import concourse.mybir as mybir
from concourse import bacc
from concourse.tile import TileContext
from concourse.masks import make_identity

F32 = mybir.dt.float32
N_CORES = 8
B, L, D, H, HD = 2, 2048, 1024, 16, 64
HPC = 4          # heads per core
A = HPC * HD     # 256 attended dims per core
SCALE = HD ** -0.5

AF = mybir.ActivationFunctionType
ALU = mybir.AluOpType

# Matmul input dtype: float32r reads the same fp32 bytes but runs the PE at
# full rate (1 cycle/row vs 4 for true fp32, which is emitted as two
# half-speed hi/lo passes). Set to mybir.dt.float32 for exact fp32.
MM_DT = mybir.dt.float32r


def _mm(nc, out, lhsT, rhs, **kw):
    nc.tensor.matmul(out, lhsT=lhsT, rhs=rhs, **kw)


def build_nc():
    nc = bacc.Bacc(trn_type="TRN2", num_devices=N_CORES, debug=False)

    xq = nc.dram_tensor("xq", (L, D), F32, kind="ExternalInput")
    xk = nc.dram_tensor("xk", (L, D), F32, kind="ExternalInput")
    xv = nc.dram_tensor("xv", (L, D), F32, kind="ExternalInput")
    wq = nc.dram_tensor("wq", (A, D), F32, kind="ExternalInput")
    wk = nc.dram_tensor("wk", (A, D), F32, kind="ExternalInput")
    wv = nc.dram_tensor("wv", (A, D), F32, kind="ExternalInput")
    wo = nc.dram_tensor("wo", (D, A), F32, kind="ExternalInput")

    out_p = nc.dram_tensor("out_p", (L, D), F32, kind="ExternalOutput")
    wsum = nc.dram_tensor("wsum", (L, L), F32, kind="ExternalOutput")

    with TileContext(nc) as tc:
        _build_tile(nc, tc, xq, xk, xv, wq, wk, wv, wo, out_p, wsum)
    nc.compile()
    return nc


def _build_tile(nc, tc, xq, xk, xv, wq, wk, wv, wo, out_p, wsum):
    DK = D // 128          # 8 d-chunks
    LC = L // 512          # 4 l-chunks
    KC = L // 128          # 16 kpos chunks
    QC = L // 1024         # 2 q super-chunks
    QS = 1024 // 128       # 8 q sub chunks per super-chunk

    with tc.tile_pool(name="const", bufs=1) as const_pool:
        ident = const_pool.tile([128, 128], F32)
        make_identity(nc, ident)

        # persistent activations
        qT = const_pool.tile([128, 2, L], MM_DT, tag="qT")      # [a%128, a//128, l]
        kT = const_pool.tile([128, 2, L], MM_DT, tag="kT")
        vv = const_pool.tile([128, KC, HPC, HD + 1], MM_DT, tag="vv")
        attT = const_pool.tile([128, 2, L], MM_DT, tag="attT")
        woT = const_pool.tile([128, 2, D], MM_DT, tag="woT")    # [a%128, a//128, o]

        # ones column for denominators (memset doesn't accept fp32r: write
        # the fp32 bit pattern through a bitcast view; 1.0 is identical)
        nc.any.memset(vv[:, :, :, HD:HD + 1].bitcast(F32), 1.0)

        # ---- Phases W+X: weight/activation transposes + projections ----
        wT_pool = tc.alloc_tile_pool(name="wT", bufs=1)
        wqT = wT_pool.tile([128, DK, A], MM_DT, tag="wqT")      # [d%128, d//128, a]
        wkT = wT_pool.tile([128, DK, A], MM_DT, tag="wkT")
        wvT = wT_pool.tile([128, DK, A], MM_DT, tag="wvT")

        with (
            tc.tile_pool(name="wload", bufs=2) as wload,
            tc.tile_pool(name="wpsum", bufs=2, space="PSUM") as wpsum,
        ):
            for w_dram, wT in ((wq, wqT), (wk, wkT), (wv, wvT)):
                w_sb = wload.tile([128, 2, D], F32, tag="w_sb")
                nc.sync.dma_start(w_sb[:], w_dram.ap().rearrange("(o p) d -> p o d", p=128))
                for dk in range(DK):
                    ps = wpsum.tile([128, 256], F32, tag="wps")
                    for o in range(2):
                        nc.tensor.transpose(
                            ps[:, o * 128:(o + 1) * 128],
                            w_sb[:, o, dk * 128:(dk + 1) * 128],
                            ident,
                        )
                    nc.vector.tensor_copy(wT[:, dk, :], ps[:])
            wo_sb = wload.tile([128, DK, A], F32, tag="wo_sb")
            nc.sync.dma_start(wo_sb[:], wo.ap().rearrange("(o p) a -> p o a", p=128))
            for a in range(2):
                for o4 in range(2):
                    ps = wpsum.tile([128, 4, 128], F32, tag="wops")
                    for oi in range(4):
                        o = o4 * 4 + oi
                        nc.tensor.transpose(
                            ps[:, oi, :],
                            wo_sb[:, o, a * 128:(a + 1) * 128],
                            ident,
                        )
                    nc.vector.tensor_copy(
                        woT[:, a, o4 * 512:(o4 + 1) * 512],
                        ps[:].rearrange("p o i -> p (o i)"),
                    )

        # ---- Phase X: x transposes + projections ----
        with (
            tc.tile_pool(name="xload", bufs=2) as xload,
            tc.tile_pool(name="xtp", bufs=2) as xtp,
            tc.tile_pool(name="xpsum", bufs=3, space="PSUM") as xpsum,
            tc.tile_pool(name="ppsum", bufs=2, space="PSUM") as ppsum,
        ):
            for lc in range(LC):
                lsl = slice(lc * 512, (lc + 1) * 512)
                for ti, (x_dram, kind) in enumerate(((xq, "q"), (xk, "k"), (xv, "v"))):
                    x_sb = xload.tile([128, 4, D], F32, tag="x_sb")
                    nc.sync.dma_start(
                        x_sb[:], x_dram.ap()[lsl, :].rearrange("(ls p) d -> p ls d", p=128)
                    )
                    xT = xtp.tile([128, DK, 512], MM_DT, tag="xT")
                    for dk in range(DK):
                        ps = xpsum.tile([128, 512], F32, tag="xps")
                        for ls in range(4):
                            nc.tensor.transpose(
                                ps[:, ls * 128:(ls + 1) * 128],
                                x_sb[:, ls, dk * 128:(dk + 1) * 128],
                                ident,
                            )
                        nc.vector.tensor_copy(xT[:, dk, :], ps[:])
                    if kind in ("q", "k"):
                        wT = wqT if kind == "q" else wkT
                        dstT = qT if kind == "q" else kT
                        for m in range(2):
                            ps = ppsum.tile([128, 512], F32, tag="pps")
                            for dk in range(DK):
                                _mm(
                                    nc,
                                    ps[:],
                                    wT[:, dk, m * 128:(m + 1) * 128],
                                    xT[:, dk, :],
                                    start=(dk == 0),
                                    stop=(dk == DK - 1),
                                )
                            if kind == "q":
                                nc.vector.tensor_scalar_mul(dstT[:, m, lsl], ps[:], SCALE)
                            else:
                                nc.vector.tensor_copy(dstT[:, m, lsl], ps[:])
                    else:
                        for kp in range(4):
                            kc = lc * 4 + kp
                            ps = ppsum.tile([128, 256], F32, tag="vps")
                            for dk in range(DK):
                                _mm(
                                    nc,
                                    ps[:],
                                    xT[:, dk, kp * 128:(kp + 1) * 128],
                                    wvT[:, dk, :],
                                    start=(dk == 0),
                                    stop=(dk == DK - 1),
                                )
                            nc.vector.tensor_copy(
                                vv[:, kc, :, 0:HD],
                                ps[:].rearrange("p (h d) -> p h d", h=HPC),
                            )
        wT_pool.release()

        # ---- Phase A: attention ----
        with (
            tc.tile_pool(name="spsum", bufs=2, space="PSUM") as spsum,
            tc.tile_pool(name="apsum", bufs=1, space="PSUM") as apsum,
            tc.tile_pool(name="npsum", bufs=2, space="PSUM") as npsum,
            tc.tile_pool(name="epool", bufs=4) as epool,
            tc.tile_pool(name="aupool", bufs=5) as aupool,
            tc.tile_pool(name="small", bufs=2) as small,
            tc.tile_pool(name="dram", bufs=8, space="DRAM") as dram_pool,
            tc.tile_pool(name="wacc", bufs=2) as wacc_pool,
            tc.tile_pool(name="opool", bufs=2) as opool,
        ):
            lse_nat = const_pool.tile([128, HPC, QC, QS], F32, tag="lse_nat")
            for qc in range(QC):
                qsl = slice(qc * 1024, (qc + 1) * 1024)
                att_uns = []
                for h in range(HPC):
                    hr = slice(64 * (h % 2), 64 * (h % 2) + 64)
                    ho = h // 2
                    att_ps = apsum.tile([65, 1024], F32, tag="att")
                    for kc in range(KC):
                        s_ps = spsum.tile([128, 1024], F32, tag="sT")
                        for half in range(2):
                            _mm(
                                nc,
                                s_ps[:, half * 512:(half + 1) * 512],
                                kT[hr, ho, kc * 128:(kc + 1) * 128],
                                qT[hr, ho, qc * 1024 + half * 512: qc * 1024 + (half + 1) * 512],
                                start=True,
                                stop=True,
                            )
                        eT = epool.tile([128, 1024], MM_DT, tag="eT")
                        nc.scalar.activation(eT[:], s_ps[:], AF.Exp)
                        for half in range(2):
                            _mm(
                                nc,
                                att_ps[:, half * 512:(half + 1) * 512],
                                vv[:, kc, h, :],
                                eT[:, half * 512:(half + 1) * 512],
                                start=(kc == 0),
                                stop=(kc == KC - 1),
                            )
                    # evict immediately with a plain copy so att_ps frees for
                    # the next head; the recip/normalize chain runs off the
                    # critical path from SBUF
                    att_un = aupool.tile([65, 1024], F32, tag="att_un")
                    nc.vector.tensor_copy(att_un[:], att_ps[:])
                    att_uns.append(att_un)
                for h in range(HPC):
                    hr = slice(64 * (h % 2), 64 * (h % 2) + 64)
                    ho = h // 2
                    att_un = att_uns[h]
                    # reciprocal must stay on partition 64 (lanes are wired
                    # to partitions; no cross-partition moves on DVE/ACT)
                    recip_f = small.tile([65, 1024], F32, tag="recip_f")
                    nc.vector.reciprocal(recip_f[64:65, :], att_un[64:65, :])
                    scratch = dram_pool.tile([1024], F32, tag="scratch")
                    nc.sync.dma_start(scratch[:][None, :], recip_f[64:65, :])
                    # replicate across 64 partitions
                    rb = small.tile([64, 1024], F32, tag="rb")
                    nc.sync.dma_start(
                        rb[:],
                        scratch[:][None, :].to_broadcast((64, 1024)),
                    )
                    recip_nat = small.tile([128, QS], F32, tag="recip_nat")
                    nc.sync.dma_start(
                        recip_nat[:], scratch[:].rearrange("(s p) -> p s", p=128)
                    )
                    nc.scalar.activation(lse_nat[:, h, qc, :], recip_nat[:], AF.Ln)
                    # normalize attended rows and store to attT; odd heads
                    # land at partitions 64-127, which DVE cannot write from
                    # a base-0 source, so bounce through SBUF via DMA
                    if h % 2 == 0:
                        nc.vector.tensor_tensor(
                            attT[0:64, ho, qsl],
                            att_un[0:64, :],
                            rb[:],
                            ALU.mult,
                        )
                    else:
                        att_tmp = small.tile([64, 1024], MM_DT, tag="att_tmp")
                        nc.vector.tensor_tensor(
                            att_tmp[:],
                            att_un[0:64, :],
                            rb[:],
                            ALU.mult,
                        )
                        nc.sync.dma_start(attT[64:128, ho, qsl], att_tmp[:])

                # natural pass: normalized softmax weights summed over heads
                for qs in range(QS):
                    q0 = qc * 1024 + qs * 128
                    wacc = wacc_pool.tile([128, L], F32, tag="wacc")
                    wtmp = wacc_pool.tile([128, L], F32, tag="wtmp")
                    for h in range(HPC):
                        hr = slice(64 * (h % 2), 64 * (h % 2) + 64)
                        ho = h // 2
                        dst = wacc if h == 0 else wtmp
                        for kq in range(4):
                            k0 = kq * 512
                            n_ps = npsum.tile([128, 512], F32, tag="nat")
                            _mm(
                                nc,
                                n_ps[:],
                                qT[hr, ho, q0:q0 + 128],
                                kT[hr, ho, k0:k0 + 512],
                                start=True,
                                stop=True,
                            )
                            nc.scalar.activation(
                                dst[:, k0:k0 + 512],
                                n_ps[:],
                                AF.Exp,
                                bias=lse_nat[:, h, qc, qs:qs + 1],
                            )
                        if h > 0:
                            nc.vector.tensor_tensor(wacc[:], wacc[:], wtmp[:], ALU.add)
                    nc.sync.dma_start(wsum.ap()[q0:q0 + 128, :], wacc[:])

                # output projection for this q super-chunk
                for qs in range(QS):
                    q0 = qc * 1024 + qs * 128
                    o_sb = opool.tile([128, 1024], F32, tag="o_sb")
                    for oh in range(2):
                        o_ps = npsum.tile([128, 512], F32, tag="nat")
                        for ac in range(2):
                            _mm(
                                nc,
                                o_ps[:],
                                attT[:, ac, q0:q0 + 128],
                                woT[:, ac, oh * 512:(oh + 1) * 512],
                                start=(ac == 0),
                                stop=(ac == 1),
                            )
                        nc.vector.tensor_copy(o_sb[:, oh * 512:(oh + 1) * 512], o_ps[:])
                    nc.sync.dma_start(out_p.ap()[q0:q0 + 128, :], o_sb[:])


_NC_CACHE = None


def _get_nc():
    global _NC_CACHE
    if _NC_CACHE is None:
        _NC_CACHE = build_nc()
    return _NC_CACHE


def make_in_maps(inputs):
    query, key, value = inputs["query"], inputs["key"], inputs["value"]
    Wq, Wk, Wv, Wo = inputs["Wq"], inputs["Wk"], inputs["Wv"], inputs["Wo"]
    in_maps = []
    for c in range(N_CORES):
        b, g = c // 4, c % 4
        asl = slice(g * A, (g + 1) * A)
        in_maps.append({
            "xq": np.ascontiguousarray(query[b], dtype=np.float32),
            "xk": np.ascontiguousarray(key[b], dtype=np.float32),
            "xv": np.ascontiguousarray(value[b], dtype=np.float32),
            "wq": np.ascontiguousarray(Wq[asl], dtype=np.float32),
            "wk": np.ascontiguousarray(Wk[asl], dtype=np.float32),
            "wv": np.ascontiguousarray(Wv[asl], dtype=np.float32),
            "wo": np.ascontiguousarray(Wo[:, asl], dtype=np.float32),
        })
    return in_maps


def kernel(query, key, value, Wq, bq, Wk, bk, Wv, bv, Wo, bo):
    from concourse.bass_utils import run_bass_kernel_spmd

    nc = _get_nc()
    in_maps = make_in_maps(dict(
        query=np.asarray(query), key=np.asarray(key), value=np.asarray(value),
        Wq=np.asarray(Wq), Wk=np.asarray(Wk), Wv=np.asarray(Wv), Wo=np.asarray(Wo),
    ))
    res = run_bass_kernel_spmd(nc, in_maps, core_ids=list(range(N_CORES)))
    out = np.zeros((B, L, D), np.float32)
    wmean = np.zeros((B, L, L), np.float32)
    for c in range(N_CORES):
        b = c // 4
        out[b] += res.results[c]["out_p"]
        wmean[b] += res.results[c]["wsum"]
    out += np.asarray(bo, np.float32)
    wmean /= H
    return out, wmean


# revision 23
# speedup vs baseline: 1.0524x; 1.0029x over previous
"""Multi-head attention (B=2, L=2048, D=1024, H=16) on 8 Trainium2 NeuronCores.

Sharding: core c -> batch b = c//4, head-group g = c%4 (heads 4g..4g+3).
Each core computes, for its batch and 4 heads:
  - partial output slice:  att_norm(heads) @ Wo[:, 256g:256g+256].T   [2048, 1024]
  - sum over its 4 heads of softmax weights                            [2048, 2048]
Host sums partials over the 4 cores of each batch (+bo) and divides the
weight sums by H=16.

Device pipeline per core (all fp32):
  1. PE-transpose Wq/Wk/Wv slices -> WqT/WkT/WvT [1024(8x128), 256];
     Wo slice -> WoT [256(2x128), 1024].
  2. Per 512-row chunk of x: PE-transpose x -> xT [1024(8x128), 512];
     project: QT/KT [256, 2048] (Q scaled by 1/8), V [2048, 4, 64] (+ones col).
  3. Per (head h, q-chunk qc of 1024):
     scores^T tiles [128kpos, 1024q] -> exp (ACT) -> attended accumulation
     with a ones-column in V giving the softmax denominator for free:
       attT_un [64, 1024], denom [1, 1024].
     recip = 1/denom; bounce recip through DRAM to get
       rb [64, 1024] (replicated) and recip_nat [128, 8] -> log -> -lse.
     attT = attT_un * rb  (normalized, stored [256, 2048] across heads).
  4. Natural-layout pass: scores [128q, 2048kpos] -> exp(S - lse) via ACT
     per-partition bias = normalized softmax weights; accumulate 4 heads
     (DVE adds) -> wsum DRAM.
  5. Output projection: out[q, :] = sum_a attT[a, q] * WoT[a, :].
"""

import numpy as np

import concourse.bass as bass
import concourse.mybir as mybir
from concourse import bacc
from concourse.tile import TileContext
from concourse.masks import make_identity

F32 = mybir.dt.float32
N_CORES = 8
B, L, D, H, HD = 2, 2048, 1024, 16, 64
HPC = 4          # heads per core
A = HPC * HD     # 256 attended dims per core
SCALE = HD ** -0.5

AF = mybir.ActivationFunctionType
ALU = mybir.AluOpType

# Matmul input dtype: float32r reads the same fp32 bytes but runs the PE at
# full rate (1 cycle/row vs 4 for true fp32, which is emitted as two
# half-speed hi/lo passes). Set to mybir.dt.float32 for exact fp32.
MM_DT = mybir.dt.float32r


def _mm(nc, out, lhsT, rhs, **kw):
    nc.tensor.matmul(out, lhsT=lhsT, rhs=rhs, **kw)


def build_nc():
    nc = bacc.Bacc(trn_type="TRN2", num_devices=N_CORES, debug=False)

    xq = nc.dram_tensor("xq", (L, D), F32, kind="ExternalInput")
    xk = nc.dram_tensor("xk", (L, D), F32, kind="ExternalInput")
    xv = nc.dram_tensor("xv", (L, D), F32, kind="ExternalInput")
    wq = nc.dram_tensor("wq", (A, D), F32, kind="ExternalInput")
    wk = nc.dram_tensor("wk", (A, D), F32, kind="ExternalInput")
    wv = nc.dram_tensor("wv", (A, D), F32, kind="ExternalInput")
    wo = nc.dram_tensor("wo", (D, A), F32, kind="ExternalInput")

    out_p = nc.dram_tensor("out_p", (L, D), F32, kind="ExternalOutput")
    wsum = nc.dram_tensor("wsum", (L, L), F32, kind="ExternalOutput")

    with TileContext(nc) as tc:
        _build_tile(nc, tc, xq, xk, xv, wq, wk, wv, wo, out_p, wsum)
    nc.compile()
    return nc


def _build_tile(nc, tc, xq, xk, xv, wq, wk, wv, wo, out_p, wsum):
    DK = D // 128          # 8 d-chunks
    LC = L // 512          # 4 l-chunks
    KC = L // 128          # 16 kpos chunks
    QC = L // 1024         # 2 q super-chunks
    QS = 1024 // 128       # 8 q sub chunks per super-chunk

    with tc.tile_pool(name="const", bufs=1) as const_pool:
        ident = const_pool.tile([128, 128], F32)
        make_identity(nc, ident)

        # persistent activations
        qT = const_pool.tile([128, 2, L], MM_DT, tag="qT")      # [a%128, a//128, l]
        kT = const_pool.tile([128, 2, L], MM_DT, tag="kT")
        vv = const_pool.tile([128, KC, HPC, HD + 1], MM_DT, tag="vv")
        attT = const_pool.tile([128, 2, L], MM_DT, tag="attT")
        woT = const_pool.tile([128, 2, D], MM_DT, tag="woT")    # [a%128, a//128, o]

        # ones column for denominators (memset doesn't accept fp32r: write
        # the fp32 bit pattern through a bitcast view; 1.0 is identical)
        nc.any.memset(vv[:, :, :, HD:HD + 1].bitcast(F32), 1.0)

        # ---- Phases W+X: weight/activation transposes + projections ----
        wT_pool = tc.alloc_tile_pool(name="wT", bufs=1)
        wqT = wT_pool.tile([128, DK, A], MM_DT, tag="wqT")      # [d%128, d//128, a]
        wkT = wT_pool.tile([128, DK, A], MM_DT, tag="wkT")
        wvT = wT_pool.tile([128, DK, A], MM_DT, tag="wvT")

        with (
            tc.tile_pool(name="wload", bufs=2) as wload,
            tc.tile_pool(name="wpsum", bufs=2, space="PSUM") as wpsum,
        ):
            for w_dram, wT in ((wq, wqT), (wk, wkT), (wv, wvT)):
                w_sb = wload.tile([128, 2, D], F32, tag="w_sb")
                nc.sync.dma_start(w_sb[:], w_dram.ap().rearrange("(o p) d -> p o d", p=128))
                for dk in range(DK):
                    ps = wpsum.tile([128, 256], F32, tag="wps")
                    for o in range(2):
                        nc.tensor.transpose(
                            ps[:, o * 128:(o + 1) * 128],
                            w_sb[:, o, dk * 128:(dk + 1) * 128],
                            ident,
                        )
                    nc.vector.tensor_copy(wT[:, dk, :], ps[:])
            wo_sb = wload.tile([128, DK, A], F32, tag="wo_sb")
            nc.sync.dma_start(wo_sb[:], wo.ap().rearrange("(o p) a -> p o a", p=128))
            for a in range(2):
                for o4 in range(2):
                    ps = wpsum.tile([128, 4, 128], F32, tag="wops")
                    for oi in range(4):
                        o = o4 * 4 + oi
                        nc.tensor.transpose(
                            ps[:, oi, :],
                            wo_sb[:, o, a * 128:(a + 1) * 128],
                            ident,
                        )
                    nc.vector.tensor_copy(
                        woT[:, a, o4 * 512:(o4 + 1) * 512],
                        ps[:].rearrange("p o i -> p (o i)"),
                    )

        # ---- Phase X: x transposes + projections ----
        with (
            tc.tile_pool(name="xload", bufs=2) as xload,
            tc.tile_pool(name="xtp", bufs=2) as xtp,
            tc.tile_pool(name="xpsum", bufs=3, space="PSUM") as xpsum,
            tc.tile_pool(name="ppsum", bufs=2, space="PSUM") as ppsum,
        ):
            for lc in range(LC):
                lsl = slice(lc * 512, (lc + 1) * 512)
                for ti, (x_dram, kind) in enumerate(((xk, "k"), (xv, "v"), (xq, "q"))):
                    x_sb = xload.tile([128, 4, D], F32, tag="x_sb")
                    nc.sync.dma_start(
                        x_sb[:], x_dram.ap()[lsl, :].rearrange("(ls p) d -> p ls d", p=128)
                    )
                    xT = xtp.tile([128, DK, 512], MM_DT, tag="xT")
                    for dk in range(DK):
                        ps = xpsum.tile([128, 512], F32, tag="xps")
                        for ls in range(4):
                            nc.tensor.transpose(
                                ps[:, ls * 128:(ls + 1) * 128],
                                x_sb[:, ls, dk * 128:(dk + 1) * 128],
                                ident,
                            )
                        nc.vector.tensor_copy(xT[:, dk, :], ps[:])
                    if kind in ("q", "k"):
                        wT = wqT if kind == "q" else wkT
                        dstT = qT if kind == "q" else kT
                        for m in range(2):
                            ps = ppsum.tile([128, 512], F32, tag="pps")
                            for dk in range(DK):
                                _mm(
                                    nc,
                                    ps[:],
                                    wT[:, dk, m * 128:(m + 1) * 128],
                                    xT[:, dk, :],
                                    start=(dk == 0),
                                    stop=(dk == DK - 1),
                                )
                            if kind == "q":
                                nc.vector.tensor_scalar_mul(dstT[:, m, lsl], ps[:], SCALE)
                            else:
                                nc.vector.tensor_copy(dstT[:, m, lsl], ps[:])
                    else:
                        for kp in range(4):
                            kc = lc * 4 + kp
                            ps = ppsum.tile([128, 256], F32, tag="vps")
                            for dk in range(DK):
                                _mm(
                                    nc,
                                    ps[:],
                                    xT[:, dk, kp * 128:(kp + 1) * 128],
                                    wvT[:, dk, :],
                                    start=(dk == 0),
                                    stop=(dk == DK - 1),
                                )
                            nc.vector.tensor_copy(
                                vv[:, kc, :, 0:HD],
                                ps[:].rearrange("p (h d) -> p h d", h=HPC),
                            )
        wT_pool.release()

        # ---- Phase A: attention ----
        with (
            tc.tile_pool(name="spsum", bufs=2, space="PSUM") as spsum,
            tc.tile_pool(name="apsum", bufs=1, space="PSUM") as apsum,
            tc.tile_pool(name="npsum", bufs=2, space="PSUM") as npsum,
            tc.tile_pool(name="epool", bufs=4) as epool,
            tc.tile_pool(name="aupool", bufs=5) as aupool,
            tc.tile_pool(name="small", bufs=2) as small,
            tc.tile_pool(name="dram", bufs=8, space="DRAM") as dram_pool,
            tc.tile_pool(name="wacc", bufs=2) as wacc_pool,
            tc.tile_pool(name="opool", bufs=2) as opool,
        ):
            lse_nat = const_pool.tile([128, HPC, QC, QS], F32, tag="lse_nat")
            for qc in range(QC):
                qsl = slice(qc * 1024, (qc + 1) * 1024)
                att_uns = []
                for h in range(HPC):
                    hr = slice(64 * (h % 2), 64 * (h % 2) + 64)
                    ho = h // 2
                    att_ps = apsum.tile([65, 1024], F32, tag="att")
                    for kc in range(KC):
                        s_ps = spsum.tile([128, 1024], F32, tag="sT")
                        for half in range(2):
                            _mm(
                                nc,
                                s_ps[:, half * 512:(half + 1) * 512],
                                kT[hr, ho, kc * 128:(kc + 1) * 128],
                                qT[hr, ho, qc * 1024 + half * 512: qc * 1024 + (half + 1) * 512],
                                start=True,
                                stop=True,
                            )
                        eT = epool.tile([128, 1024], MM_DT, tag="eT")
                        nc.scalar.activation(eT[:], s_ps[:], AF.Exp)
                        for half in range(2):
                            _mm(
                                nc,
                                att_ps[:, half * 512:(half + 1) * 512],
                                vv[:, kc, h, :],
                                eT[:, half * 512:(half + 1) * 512],
                                start=(kc == 0),
                                stop=(kc == KC - 1),
                            )
                    # evict immediately with a plain copy so att_ps frees for
                    # the next head; the recip/normalize chain runs off the
                    # critical path from SBUF
                    att_un = aupool.tile([65, 1024], F32, tag="att_un")
                    nc.vector.tensor_copy(att_un[:], att_ps[:])
                    att_uns.append(att_un)
                for h in range(HPC):
                    hr = slice(64 * (h % 2), 64 * (h % 2) + 64)
                    ho = h // 2
                    att_un = att_uns[h]
                    # reciprocal must stay on partition 64 (lanes are wired
                    # to partitions; no cross-partition moves on DVE/ACT)
                    recip_f = small.tile([65, 1024], F32, tag="recip_f")
                    nc.vector.reciprocal(recip_f[64:65, :], att_un[64:65, :])
                    scratch = dram_pool.tile([1024], F32, tag="scratch")
                    nc.gpsimd.dma_start(scratch[:][None, :], recip_f[64:65, :])
                    # replicate across 64 partitions
                    rb = small.tile([64, 1024], F32, tag="rb")
                    nc.gpsimd.dma_start(
                        rb[:],
                        scratch[:][None, :].to_broadcast((64, 1024)),
                    )
                    recip_nat = small.tile([128, QS], F32, tag="recip_nat")
                    nc.gpsimd.dma_start(
                        recip_nat[:], scratch[:].rearrange("(s p) -> p s", p=128)
                    )
                    nc.scalar.activation(lse_nat[:, h, qc, :], recip_nat[:], AF.Ln)
                    # normalize attended rows and store to attT; odd heads
                    # land at partitions 64-127, which DVE cannot write from
                    # a base-0 source, so bounce through SBUF via DMA
                    if h % 2 == 0:
                        nc.vector.tensor_tensor(
                            attT[0:64, ho, qsl],
                            att_un[0:64, :],
                            rb[:],
                            ALU.mult,
                        )
                    else:
                        att_tmp = small.tile([64, 1024], MM_DT, tag="att_tmp")
                        nc.vector.tensor_tensor(
                            att_tmp[:],
                            att_un[0:64, :],
                            rb[:],
                            ALU.mult,
                        )
                        nc.gpsimd.dma_start(attT[64:128, ho, qsl], att_tmp[:])

                # natural pass: normalized softmax weights summed over heads
                for qs in range(QS):
                    q0 = qc * 1024 + qs * 128
                    wacc = wacc_pool.tile([128, L], F32, tag="wacc")
                    wtmp = wacc_pool.tile([128, L], F32, tag="wtmp")
                    for h in range(HPC):
                        hr = slice(64 * (h % 2), 64 * (h % 2) + 64)
                        ho = h // 2
                        dst = wacc if h == 0 else wtmp
                        for kq in range(4):
                            k0 = kq * 512
                            n_ps = npsum.tile([128, 512], F32, tag="nat")
                            _mm(
                                nc,
                                n_ps[:],
                                qT[hr, ho, q0:q0 + 128],
                                kT[hr, ho, k0:k0 + 512],
                                start=True,
                                stop=True,
                            )
                            nc.scalar.activation(
                                dst[:, k0:k0 + 512],
                                n_ps[:],
                                AF.Exp,
                                bias=lse_nat[:, h, qc, qs:qs + 1],
                            )
                        if h > 0:
                            nc.vector.tensor_tensor(wacc[:], wacc[:], wtmp[:], ALU.add)
                    nc.sync.dma_start(wsum.ap()[q0:q0 + 128, :], wacc[:])

                # output projection for this q super-chunk
                for qs in range(QS):
                    q0 = qc * 1024 + qs * 128
                    o_sb = opool.tile([128, 1024], F32, tag="o_sb")
                    for oh in range(2):
                        o_ps = npsum.tile([128, 512], F32, tag="nat")
                        for ac in range(2):
                            _mm(
                                nc,
                                o_ps[:],
                                attT[:, ac, q0:q0 + 128],
                                woT[:, ac, oh * 512:(oh + 1) * 512],
                                start=(ac == 0),
                                stop=(ac == 1),
                            )
                        nc.vector.tensor_copy(o_sb[:, oh * 512:(oh + 1) * 512], o_ps[:])
                    nc.sync.dma_start(out_p.ap()[q0:q0 + 128, :], o_sb[:])


_NC_CACHE = None


def _get_nc():
    global _NC_CACHE
    if _NC_CACHE is None:
        _NC_CACHE = build_nc()
    return _NC_CACHE


def make_in_maps(inputs):
    query, key, value = inputs["query"], inputs["key"], inputs["value"]
    Wq, Wk, Wv, Wo = inputs["Wq"], inputs["Wk"], inputs["Wv"], inputs["Wo"]
    in_maps = []
    for c in range(N_CORES):
        b, g = c // 4, c % 4
        asl = slice(g * A, (g + 1) * A)
        in_maps.append({
            "xq": np.ascontiguousarray(query[b], dtype=np.float32),
            "xk": np.ascontiguousarray(key[b], dtype=np.float32),
            "xv": np.ascontiguousarray(value[b], dtype=np.float32),
            "wq": np.ascontiguousarray(Wq[asl], dtype=np.float32),
            "wk": np.ascontiguousarray(Wk[asl], dtype=np.float32),
            "wv": np.ascontiguousarray(Wv[asl], dtype=np.float32),
            "wo": np.ascontiguousarray(Wo[:, asl], dtype=np.float32),
        })
    return in_maps


def kernel(query, key, value, Wq, bq, Wk, bk, Wv, bv, Wo, bo):
    from concourse.bass_utils import run_bass_kernel_spmd

    nc = _get_nc()
    in_maps = make_in_maps(dict(
        query=np.asarray(query), key=np.asarray(key), value=np.asarray(value),
        Wq=np.asarray(Wq), Wk=np.asarray(Wk), Wv=np.asarray(Wv), Wo=np.asarray(Wo),
    ))
    res = run_bass_kernel_spmd(nc, in_maps, core_ids=list(range(N_CORES)))
    out = np.zeros((B, L, D), np.float32)
    wmean = np.zeros((B, L, L), np.float32)
    for c in range(N_CORES):
        b = c // 4
        out[b] += res.results[c]["out_p"]
        wmean[b] += res.results[c]["wsum"]
    out += np.asarray(bo, np.float32)
    wmean /= H
    return out, wmean
